# revision 12
# baseline (speedup 1.0000x reference)
"""Trainium2 Bass kernel for nn_AttentionSHA (dense transformer attention block).

Full inputs -> full output. Tensor-parallel over heads across 8 NeuronCores
(core g owns kv-head g and query heads 4g..4g+3; wo row-sharded), host-side
reduce of the 8 partial output projections.

v2 (bf16): all matmul operands in bf16 (f32 PSUM accumulation), which halves
HBM traffic and SBUF footprint vs the fp32r baseline (fixes phase-1 DMA
starvation). The softmax denominator z is folded into the PV matmul as a
129th all-ones column of V: PV is computed "flipped" (stationary = expm
chunk [t,s], moving = [V | 1] [t, e+1]) so out[s, e] and z[s] land with s on
partitions, where a per-partition reciprocal scale normalizes for free.
This removes the 64 ones-matmuls (13.6us of PE) the baseline spent on z.
The V projection is emitted pre-transposed (stationary = x chunk, moving =
wv) so no V transposes are needed; att is transposed back [s,e]->[e,s] on
the PE (cheap, 128 cols each) for the wo projection.

Math notes (validated against the reference in fp64/fp32 numpy):
  - The reference adds a 0/1 causal mask *before* softmax (no -inf masking)
    and runs softmax over the full MAXSEQ=2048 cache axis where positions
    >= S hold zero k/v. Softmax without max-subtraction is exact here
    (scores are in [-17, 18]), so:
      out = sum_t exp(sc_t)*m_t*v_t / (sum_t exp(sc_t)*m_t + 1024)
    with m_t = e if visible else 1, and +1024 = (MAXSEQ - S) zero-score
    tail. The e-factor for fully-visible regions folds into the Exp bias
    (exp(x + 1) = e*exp(x)); only the 128x128 diagonal blocks need a mask
    multiply.
  - RoPE is applied via host-permuted weight rows (even channels then odd),
    a partition-half swap, and two multiply-adds against [cos;cos] /
    [-sin;sin].
"""
import numpy as np
from contextlib import ExitStack

S = 1024
D = 4096
NH = 32
NKV = 8
HD = 128
NREP = NH // NKV          # 4
MAXSEQ = 2048
NCORES = 8
DT = D // 128             # 32 d-tiles
TT = S // 128             # 8 t-tiles

_CACHE = {}


def _build_nc(phases=4, repeat=1):
    import concourse.bacc as bacc
    import concourse.mybir as mybir
    import concourse.tile as tile

    f32 = mybir.dt.float32
    bf16 = mybir.dt.bfloat16
    Exp = mybir.ActivationFunctionType.Exp
    mult = mybir.AluOpType.mult
    add = mybir.AluOpType.add

    nc = bacc.Bacc("TRN2", target_bir_lowering=False, debug=False,
                   num_devices=NCORES)

    xT = nc.dram_tensor("xT", [D, S], bf16, kind="ExternalInput")
    wq_t = nc.dram_tensor("wq_t", [NREP, 128, DT * HD], bf16, kind="ExternalInput")
    wk_t = nc.dram_tensor("wk_t", [128, DT * HD], bf16, kind="ExternalInput")
    wv_t = nc.dram_tensor("wv_t", [128, DT * HD], bf16, kind="ExternalInput")
    wo_t = nc.dram_tensor("wo_t", [NREP * HD, D], bf16, kind="ExternalInput")
    cc_d = nc.dram_tensor("cc", [HD, S], f32, kind="ExternalInput")
    ns_d = nc.dram_tensor("ns", [HD, S], f32, kind="ExternalInput")
    emaskd_d = nc.dram_tensor("emaskd", [128, TT * 128], bf16, kind="ExternalInput")
    ident_d = nc.dram_tensor("ident", [128, 128], bf16, kind="ExternalInput")
    outT = nc.dram_tensor("outT", [D, S], bf16, kind="ExternalOutput")

    with tile.TileContext(nc) as tc, ExitStack() as ctx:
        const = ctx.enter_context(tc.tile_pool(name="const", bufs=1))
        wts = ctx.enter_context(tc.tile_pool(name="wts", bufs=6))
        xpool = ctx.enter_context(tc.tile_pool(name="xpool", bufs=8))
        rpool = ctx.enter_context(tc.tile_pool(name="rpool", bufs=3))
        qkv = ctx.enter_context(tc.tile_pool(name="qkv", bufs=1))
        hs = ctx.enter_context(tc.tile_pool(name="hs", bufs=5))
        epool = ctx.enter_context(tc.tile_pool(name="epool", bufs=10))
        apool = ctx.enter_context(tc.tile_pool(name="apool", bufs=3))
        zpool = ctx.enter_context(tc.tile_pool(name="zpool", bufs=4))
        opool = ctx.enter_context(tc.tile_pool(name="opool", bufs=3))
        ps = ctx.enter_context(tc.tile_pool(name="ps", bufs=8, space="PSUM"))

        def _body():
            # ---- constants (loaded lazily at first use site) ----
            cc_sb = const.tile([128, S], f32)
            ns_sb = const.tile([128, S], f32)
            ident_sb = const.tile([128, 128], bf16)
            emaskd_sb = const.tile([128, TT * 128], bf16)

            # ---- weights; wo reuses these slots later ----
            wq_sb = [wts.tile([128, D], bf16, name=f"wq_sb{h}", tag="w16")
                     for h in range(NREP)]
            wk_sb = wts.tile([128, D], bf16, tag="w16")
            wv_sb = wts.tile([128, D], bf16, tag="w16")

            WCHUNKS = [(d, 4) for d in range(0, DT, 4)]
            _wb = {d0: (d0, ln) for d0, ln in WCHUNKS}

            def load_w_chunk_span(d0, ln):
                c0, c1 = 128 * d0, 128 * (d0 + ln)
                for h in range(NREP):
                    nc.sync.dma_start(wq_sb[h][:, c0:c1], wq_t[h][:, c0:c1])
                nc.sync.dma_start(wk_sb[:, c0:c1], wk_t[:, c0:c1])
                nc.sync.dma_start(wv_sb[:, c0:c1], wv_t[:, c0:c1])

            if phases < 1:
                nul = const.tile([128, S], bf16, name="nul")
                nc.sync.dma_start(nul[:], xT[0:128, :])
                nc.sync.dma_start(outT[0:128, :], nul[:])
                return
            # ---- phase 1: QKV projections + RoPE ----
            # q_rot/k_rot: [e, s] bf16; v_te: per t-tile [t, e | ones] bf16,
            # produced pre-transposed by flipping the V matmul operands.
            q_rot = [hs.tile([128, S], bf16, name=f"q_rot{h}", tag="hs")
                     for h in range(NREP)]
            k_rot = hs.tile([128, S], bf16, tag="hs")
            v_et = qkv.tile([128, S], bf16, name="v_et")        # [e, t]
            v_te = [qkv.tile([128, 132], bf16, name=f"v_te{t}") for t in range(TT)]
            for t in range(TT):
                nc.vector.memset(v_te[t][:, 128:129], 1.0)

            def v_transpose(t):
                # [e, t] -> [t, e] via PE; lands next to the ones column
                tr = ps.tile([128, 128], bf16, tag="ps", name="vtr")
                nc.tensor.transpose(tr[:], v_et[:, 128 * t:128 * (t + 1)],
                                    ident_sb[:])
                nc.vector.tensor_copy(v_te[t][:, 0:128], tr[:])

            for sh in range(2):
                s0 = 512 * sh
                q_ps = [ps.tile([128, 512], f32, tag="ps", name=f"q_ps{sh}_{h}")
                        for h in range(NREP)]
                k_ps = ps.tile([128, 512], f32, tag="ps", name=f"k_ps{sh}")
                v_ps = ps.tile([128, 512], f32, tag="ps", name=f"v_ps{sh}")
                for d in range(DT):
                    x_r = xpool.tile([128, 512], bf16, name="x_r")
                    nc.sync.dma_start(x_r[:], xT[128 * d:128 * (d + 1), s0:s0 + 512])
                    if sh == 0 and d in _wb:
                        load_w_chunk_span(*_wb[d])
                    for h in range(NREP):
                        nc.tensor.matmul(q_ps[h][:], wq_sb[h][:, 128 * d:128 * (d + 1)],
                                         x_r[:], start=(d == 0), stop=(d == DT - 1))
                    nc.tensor.matmul(k_ps[:], wk_sb[:, 128 * d:128 * (d + 1)],
                                     x_r[:], start=(d == 0), stop=(d == DT - 1))
                    nc.tensor.matmul(v_ps[:], wv_sb[:, 128 * d:128 * (d + 1)],
                                     x_r[:], start=(d == 0), stop=(d == DT - 1))

                if sh == 0:
                    nc.sync.dma_start(cc_sb[:], cc_d[:])
                    nc.sync.dma_start(ns_sb[:], ns_d[:])
                    nc.sync.dma_start(ident_sb[:], ident_d[:])

                # RoPE: dest = psum*[cos;cos] + swap(psum)*[-sin;sin].
                # fast=True splits the swap copies across ACT+DVE — used for
                # q0 and k, whose rope latency gates phase 3's first scores
                def rope(psum, dest, fast=False):
                    sw = rpool.tile([128, 512], f32, name="sw")
                    if fast:
                        nc.vector.tensor_copy(sw[0:64, :], psum[64:128, :])
                    else:
                        nc.scalar.copy(sw[0:64, :], psum[64:128, :])
                    nc.scalar.copy(sw[64:128, :], psum[0:64, :])
                    t1 = rpool.tile([128, 512], f32, name="t1")
                    nc.vector.tensor_tensor(t1[:], psum[:], cc_sb[:, s0:s0 + 512], op=mult)
                    t2 = rpool.tile([128, 512], f32, name="t2")
                    nc.gpsimd.tensor_tensor(t2[:], sw[:], ns_sb[:, s0:s0 + 512], op=mult)
                    nc.vector.tensor_tensor(dest, t1[:], t2[:], op=add)

                # V psum -> v_et (bf16) on ACT; sh0 tiles transposed now,
                # sh1 transposes deferred to phase-3 start (overlap the
                # first scores/exp instead of blocking behind the rope queue)
                nc.scalar.copy(v_et[:, s0:s0 + 512], v_ps[:])
                rope(q_ps[0], q_rot[0][:, s0:s0 + 512], fast=(sh == 1))
                rope(k_ps, k_rot[:, s0:s0 + 512], fast=(sh == 1))
                if sh == 0:
                    for t in range(4):
                        v_transpose(t)
                for h in range(1, NREP):
                    rope(q_ps[h], q_rot[h][:, s0:s0 + 512], fast=(sh == 1))

            if phases < 2:
                nc.sync.dma_start(outT[0:128, :], k_rot[:])
                return

            # ---- phase 3: attention per head ----
            att_t = []            # per head [e, s] bf16, normalized, transposed
            inv_sqrt_hd = float(1.0 / np.sqrt(HD))
            if phases >= 3:
                nc.sync.dma_start(emaskd_sb[:], emaskd_d[:])
                for t in range(4, TT):
                    v_transpose(t)
            # wo loads early: overlap attention phase (slots reuse wq/wk/wv)
            wo_sb = []
            for h in range(NREP if phases >= 4 else 0):
                w = wts.tile([128, D], bf16, name=f"wo_sb{h}", tag="w16")
                nc.sync.dma_start(w[:], wo_t[128 * h:128 * (h + 1), :])
                wo_sb.append(w)

            # PSUM allows only one open accumulation group per bank at a
            # time, so the 8 s-chunks run as 3 sequential waves over 3 banks:
            # wave w handles chunks [w, 3+w, 6+w (if w<2)], chunk 3g+w living
            # in oz tile g at column base 129*w.
            WAVES = [[0, 3, 6], [1, 4, 7], [2, 5]]

            for h in range(NREP if phases >= 3 else 0):
                def emit_sc_exp(t):
                    dlo, dhi = 128 * t, 128 * (t + 1)
                    expm = epool.tile([128, S], bf16, name="expm")
                    for c in range(2):
                        sc = ps.tile([128, 512], f32, tag="ps", name="sc")
                        nc.tensor.matmul(sc[:], k_rot[:, dlo:dhi],
                                         q_rot[h][:, 512 * c:512 * (c + 1)],
                                         start=True, stop=True)
                        lo, hi = 512 * c, 512 * (c + 1)
                        if dlo >= hi:
                            # fully invisible: plain exp
                            nc.scalar.activation(expm[:, lo:hi], sc[:], Exp,
                                                 scale=inv_sqrt_hd)
                        elif dhi <= lo:
                            # fully visible: exp(x + 1) = e * exp(x)
                            nc.scalar.activation(expm[:, lo:hi], sc[:], Exp,
                                                 scale=inv_sqrt_hd, bias=1.0)
                        else:
                            # diagonal block inside this chunk: one exp call,
                            # then the mask factors applied in-place (diag x
                            # emaskd on GpSimd; visible remainder x e on DVE)
                            nc.scalar.activation(expm[:, lo:hi], sc[:], Exp,
                                                 scale=inv_sqrt_hd)
                            nc.gpsimd.tensor_tensor(
                                expm[:, dlo:dhi], expm[:, dlo:dhi],
                                emaskd_sb[:, 128 * t:128 * (t + 1)], op=mult)
                            if dhi < hi:
                                nc.gpsimd.tensor_scalar_mul(
                                    expm[:, dhi:hi], expm[:, dhi:hi],
                                    float(np.e))
                    return expm

                pend = [emit_sc_exp(0), emit_sc_exp(1)]
                expms = []
                oz_w0 = [ps.tile([128, 512], f32, tag="ps", name=f"oz{h}_{c}")
                         for c in WAVES[0]]
                for t in range(TT):
                    if t + 2 < TT:
                        pend.append(emit_sc_exp(t + 2))
                    expm_t = pend.pop(0)
                    expms.append(expm_t)
                    # wave 0 runs inside the t loop, consuming exps as they
                    # arrive; waves 1-2 re-walk the (resident) expm tiles
                    for gi, c in enumerate(WAVES[0]):
                        nc.tensor.matmul(oz_w0[gi][:, 0:129],
                                         expm_t[:, 128 * c:128 * (c + 1)],
                                         v_te[t][:, 0:129],
                                         start=(t == 0), stop=(t == TT - 1))
                a = hs.tile([128, S], bf16, name=f"att_t{h}", tag="hs")

                def finish_chunk(c, oz):
                    z_sb = zpool.tile([128, 1], f32, name="z_sb")
                    nc.vector.tensor_scalar_add(z_sb[:], oz[:, 128:129],
                                                float(MAXSEQ - S))
                    rz = zpool.tile([128, 1], f32, name="rz")
                    nc.vector.reciprocal(rz[:], z_sb[:])
                    att_n = apool.tile([128, 128], bf16, name="att_n")
                    nc.vector.tensor_scalar_mul(att_n[:], oz[:, 0:128], rz[:])
                    tr = ps.tile([128, 128], bf16, tag="ps", name="tr")
                    nc.tensor.transpose(tr[:], att_n[:], ident_sb[:])
                    if c % 2 == 0:
                        nc.scalar.copy(a[:, 128 * c:128 * (c + 1)], tr[:])
                    else:
                        nc.vector.tensor_copy(a[:, 128 * c:128 * (c + 1)], tr[:])

                prev = list(zip(WAVES[0], oz_w0))
                for w in range(1, 3):
                    oz_w = [ps.tile([128, 512], f32, tag="ps", name=f"oz{h}_{c}")
                            for c in WAVES[w]]
                    for gi, c in enumerate(WAVES[w]):
                        for t in range(TT):
                            nc.tensor.matmul(oz_w[gi][:, 0:129],
                                             expms[t][:, 128 * c:128 * (c + 1)],
                                             v_te[t][:, 0:129],
                                             start=(t == 0), stop=(t == TT - 1))
                    for c, oz in prev:
                        finish_chunk(c, oz)
                    prev = list(zip(WAVES[w], oz_w))
                for c, oz in prev:
                    finish_chunk(c, oz)
                att_t.append(a)

            if phases == 3:
                for h in range(NREP):
                    nc.sync.dma_start(outT[128 * h:128 * (h + 1), :], att_t[h][:])
                return
            # ---- phase 4: output projection (partial over this core's 512 rows) ----
            for do in range(DT if phases >= 4 else 0):
                op_ps = [ps.tile([128, 512], f32, tag="ps", name=f"op{c}")
                         for c in range(2)]
                for h in range(NREP):
                    for c in range(2):
                        nc.tensor.matmul(op_ps[c][:],
                                         wo_sb[h][:, 128 * do:128 * (do + 1)],
                                         att_t[h][:, 512 * c:512 * (c + 1)],
                                         start=(h == 0), stop=(h == NREP - 1))
                out_sb = opool.tile([128, S], bf16, name="out_sb")
                nc.vector.tensor_copy(out_sb[:, 0:512], op_ps[0][:])
                nc.scalar.copy(out_sb[:, 512:1024], op_ps[1][:])
                nc.sync.dma_start(outT[128 * do:128 * (do + 1), :], out_sb[:])

        for _rep in range(repeat):
            _body()

    nc.compile()
    return nc


def kernel(**inputs):
    import ml_dtypes
    from concourse.bass_utils import run_bass_kernel_spmd
    bf = ml_dtypes.bfloat16

    x = np.asarray(inputs["x"], np.float32)                 # [1, S, D]
    cos = np.asarray(inputs["freqs_cos"], np.float32)       # [S, 64]
    sin = np.asarray(inputs["freqs_sin"], np.float32)       # [S, 64]
    wq = np.asarray(inputs["wq"], np.float32)               # [NH, HD, D]
    wk = np.asarray(inputs["wk"], np.float32)               # [NKV, HD, D]
    wv = np.asarray(inputs["wv"], np.float32)               # [NKV, HD, D]
    wo = np.asarray(inputs["wo"], np.float32)               # [D, D]
    input_pos = np.asarray(inputs["input_pos"]).astype(np.int64)  # [S]

    if "nc" not in _CACHE:
        _CACHE["nc"] = _build_nc()
    nc = _CACHE["nc"]

    perm = np.concatenate([np.arange(0, HD, 2), np.arange(1, HD, 2)])
    xT = np.ascontiguousarray(x[0].T).astype(bf)            # [D, S]
    cc = np.ascontiguousarray(np.concatenate([cos.T, cos.T], 0))   # [128, S]
    ns = np.ascontiguousarray(np.concatenate([-sin.T, sin.T], 0))  # [128, S]
    # visibility adds +1 pre-exp where input_pos[t] <= input_pos[s]; for the
    # (spec-guaranteed) sorted arange fill only diagonal blocks are mixed.
    emaskd_t = np.empty((TT, 128, 128), np.float32)
    for t in range(TT):
        p = input_pos[128 * t:128 * (t + 1)]
        emaskd_t[t] = np.where(p[:, None] <= p[None, :], np.float32(np.e),
                               np.float32(1.0))
    # partition-major [128, TT*128] so the single DMA reads contiguous runs
    emaskd = np.ascontiguousarray(
        emaskd_t.transpose(1, 0, 2).reshape(128, TT * 128)).astype(bf)
    ident = np.eye(128, dtype=np.float32).astype(bf)

    def pmajor(wT):
        # [D, 128e] -> [128p, DT*128e]: partition-major so each chunk DMA
        # reads contiguous runs per partition
        return np.ascontiguousarray(
            wT.reshape(DT, 128, HD).transpose(1, 0, 2).reshape(128, DT * HD))

    in_maps = []
    for g in range(NCORES):
        wq_g = wq[NREP * g:NREP * (g + 1)][:, perm, :]       # [4, 128, D]
        in_maps.append({
            "xT": xT,
            "wq_t": np.stack([pmajor(wq_g[j].T) for j in range(NREP)]).astype(bf),
            "wk_t": pmajor(wk[g][perm].T).astype(bf),        # [128, DT*128]
            "wv_t": pmajor(wv[g].T).astype(bf),              # [128, DT*128]
            "wo_t": np.ascontiguousarray(
                wo[:, NREP * HD * g:NREP * HD * (g + 1)].T).astype(bf),  # [512, D]
            "cc": cc, "ns": ns, "emaskd": emaskd, "ident": ident,
        })

    res = run_bass_kernel_spmd(nc, in_maps, list(range(NCORES)))
    total = np.zeros((D, S), np.float64)
    for g in range(NCORES):
        total += res.results[g]["outT"].astype(np.float64)
    return np.ascontiguousarray(total.T.astype(np.float32)[None])   # [1, S, D]


# revision 23
# speedup vs baseline: 1.0833x; 1.0833x over previous
"""Trainium2 Bass kernel for nn_AttentionSHA (dense transformer attention block).

Full inputs -> full output. Tensor-parallel over heads across 8 NeuronCores
(core g owns kv-head g and query heads 4g..4g+3; wo row-sharded), host-side
reduce of the 8 partial output projections.

v3 (bf16 + software-pipelined schedule):
  - all matmul operands bf16 (f32 PSUM accumulation): halves HBM traffic and
    SBUF footprint vs fp32r; x is kept resident in SBUF (16 DMAs of 4KB/
    partition) so the lead phase computes only K, V and Q0.
  - the Q1..Q3 projections are drip-fed through the attention slots of the
    preceding heads, giving the PE independent work that hides the ACT exp
    stream (9.1us/head) and the DVE normalize chain.
  - softmax denominator z is folded into the PV matmul as a 129th all-ones
    column of V: PV runs "flipped" (stationary = expm s-chunk [t,s], moving
    = [V|1] [t,129]) so out[s,e] and z[s] land with s on partitions, where a
    per-partition reciprocal scale normalizes for free. att is transposed
    back [s,e]->[e,s] on the PE (128 cols each) for the wo projection.
    PSUM allows only one open accumulation group per bank, so the 8 s-chunks
    run as 4 sequential waves of 2 full-bank tiles.
  - during the last head's attention, the first wo output tiles pre-
    accumulate heads 0..2 (closed in phase C), filling the tail bubble.

Math notes (validated against the reference in fp64/fp32 numpy):
  - The reference adds a 0/1 causal mask *before* softmax (no -inf masking)
    and runs softmax over the full MAXSEQ=2048 cache axis where positions
    >= S hold zero k/v. Softmax without max-subtraction is exact here
    (scores are in [-17, 18]), so:
      out = sum_t exp(sc_t)*m_t*v_t / (sum_t exp(sc_t)*m_t + 1024)
    with m_t = e if visible else 1, and +1024 = (MAXSEQ - S) zero-score
    tail. The e-factor for fully-visible regions folds into the Exp bias
    (exp(x + 1) = e*exp(x)); only the 128x128 diagonal blocks need a mask
    multiply.
  - RoPE is applied via host-permuted weight rows (even channels then odd),
    a partition-half swap, and two multiply-adds against [cos;cos] /
    [-sin;sin].
"""
import numpy as np
from collections import deque
from contextlib import ExitStack

S = 1024
D = 4096
NH = 32
NKV = 8
HD = 128
NREP = NH // NKV          # 4
MAXSEQ = 2048
NCORES = 8
DT = D // 128             # 32 d-tiles
TT = S // 128             # 8 t-tiles
NPRE = 2                  # wo tiles pre-accumulated during the last head

_CACHE = {}


def _build_nc(phases=4, repeat=1):
    import concourse.bacc as bacc
    import concourse.mybir as mybir
    import concourse.tile as tile

    f32 = mybir.dt.float32
    bf16 = mybir.dt.bfloat16
    Exp = mybir.ActivationFunctionType.Exp
    mult = mybir.AluOpType.mult
    add = mybir.AluOpType.add

    nc = bacc.Bacc("TRN2", target_bir_lowering=False, debug=False,
                   num_devices=NCORES)

    # x host-packed partition-major: col d*1024 + sh*512 + s
    xp = nc.dram_tensor("xp", [128, DT * S], bf16, kind="ExternalInput")
    wq_t = nc.dram_tensor("wq_t", [NREP, 128, DT * HD], bf16, kind="ExternalInput")
    wk_t = nc.dram_tensor("wk_t", [128, DT * HD], bf16, kind="ExternalInput")
    wv_t = nc.dram_tensor("wv_t", [128, DT * HD], bf16, kind="ExternalInput")
    wo_t = nc.dram_tensor("wo_t", [NREP * HD, D], bf16, kind="ExternalInput")
    cc_d = nc.dram_tensor("cc", [HD, S], f32, kind="ExternalInput")
    ns_d = nc.dram_tensor("ns", [HD, S], f32, kind="ExternalInput")
    emaskd_d = nc.dram_tensor("emaskd", [128, TT * 128], bf16, kind="ExternalInput")
    ident_d = nc.dram_tensor("ident", [128, 128], bf16, kind="ExternalInput")
    outT = nc.dram_tensor("outT", [D, S], bf16, kind="ExternalOutput")

    with tile.TileContext(nc) as tc, ExitStack() as ctx:
        const = ctx.enter_context(tc.tile_pool(name="const", bufs=1))
        wts = ctx.enter_context(tc.tile_pool(name="wts", bufs=6))
        xpool = ctx.enter_context(tc.tile_pool(name="xpool", bufs=1))
        rpool = ctx.enter_context(tc.tile_pool(name="rpool", bufs=3))
        qkv = ctx.enter_context(tc.tile_pool(name="qkv", bufs=1))
        hs = ctx.enter_context(tc.tile_pool(name="hs", bufs=5))
        epool = ctx.enter_context(tc.tile_pool(name="epool", bufs=12))
        apool = ctx.enter_context(tc.tile_pool(name="apool", bufs=3))
        zpool = ctx.enter_context(tc.tile_pool(name="zpool", bufs=4))
        opool = ctx.enter_context(tc.tile_pool(name="opool", bufs=3))
        ps = ctx.enter_context(tc.tile_pool(name="ps", bufs=8, space="PSUM"))

        def _body():
            cc_sb = const.tile([128, S], f32)
            ns_sb = const.tile([128, S], f32)
            ident_sb = const.tile([128, 128], bf16)
            emaskd_sb = const.tile([128, TT * 128], bf16)

            wq_sb = [wts.tile([128, D], bf16, name=f"wq_sb{h}", tag="w16")
                     for h in range(NREP)]
            wk_sb = wts.tile([128, D], bf16, tag="w16")
            wv_sb = wts.tile([128, D], bf16, tag="w16")

            # ---- resident x: one big tile, DMAd in 16 slices ----
            xres = xpool.tile([128, DT * S], bf16, name="xres")

            def xsl(d, sh):
                return xres[:, 1024 * d + 512 * sh:1024 * d + 512 * sh + 512]

            for i in range(2):
                nc.sync.dma_start(xres[:, 2048 * i:2048 * (i + 1)],
                                  xp[:, 2048 * i:2048 * (i + 1)])
            nc.sync.dma_start(wk_sb[:, 0:1024], wk_t[:, 0:1024])
            nc.sync.dma_start(wv_sb[:, 0:1024], wv_t[:, 0:1024])
            nc.sync.dma_start(wq_sb[0][:, 0:1024], wq_t[0][:, 0:1024])

            if phases < 1:
                nc.sync.dma_start(outT[0:128, :], xres[:, 0:1024])
                return

            q_rot = [hs.tile([128, S], bf16, name=f"q_rot{h}", tag="hs")
                     for h in range(NREP)]
            k_rot = hs.tile([128, S], bf16, tag="hs")
            v_et = qkv.tile([128, S], bf16, name="v_et")        # [e, t]
            v_te = [qkv.tile([128, 132], bf16, name=f"v_te{t}") for t in range(TT)]
            for t in range(TT):
                nc.vector.memset(v_te[t][:, 128:129], 1.0)

            def v_transpose(t):
                tr = ps.tile([128, 128], bf16, tag="ps", name="vtr")
                nc.tensor.transpose(tr[:], v_et[:, 128 * t:128 * (t + 1)],
                                    ident_sb[:])
                nc.vector.tensor_copy(v_te[t][:, 0:128], tr[:])

            # RoPE: dest = psum*[cos;cos] + swap(psum)*[-sin;sin]
            def rope(psum, dest, s0, fast=False):
                sw = rpool.tile([128, 512], f32, name="sw")
                if fast:
                    nc.vector.tensor_copy(sw[0:64, :], psum[64:128, :])
                else:
                    nc.scalar.copy(sw[0:64, :], psum[64:128, :])
                nc.scalar.copy(sw[64:128, :], psum[0:64, :])
                t1 = rpool.tile([128, 512], f32, name="t1")
                nc.vector.tensor_tensor(t1[:], psum[:], cc_sb[:, s0:s0 + 512], op=mult)
                t2 = rpool.tile([128, 512], f32, name="t2")
                nc.gpsimd.tensor_tensor(t2[:], sw[:], ns_sb[:, s0:s0 + 512], op=mult)
                nc.vector.tensor_tensor(dest, t1[:], t2[:], op=add)

            # ---- phase A: K, V, Q0 (+ prefetch weights) ----
            k_ps = [ps.tile([128, 512], f32, tag="ps", name=f"k_ps{sh}")
                    for sh in range(2)]
            v_ps = [ps.tile([128, 512], f32, tag="ps", name=f"v_ps{sh}")
                    for sh in range(2)]
            q0_ps = [ps.tile([128, 512], f32, tag="ps", name=f"q0_ps{sh}")
                     for sh in range(2)]
            for d in range(DT):
                if d % 2 == 0 and d // 2 + 2 < 16:
                    i = d // 2 + 2
                    nc.sync.dma_start(xres[:, 2048 * i:2048 * (i + 1)],
                                      xp[:, 2048 * i:2048 * (i + 1)])
                if d % 4 == 0 and d < 24:
                    c0, c1 = 1024 + 512 * (d // 4), 1024 + 512 * (d // 4 + 1)
                    nc.sync.dma_start(wk_sb[:, c0:c1], wk_t[:, c0:c1])
                    nc.sync.dma_start(wv_sb[:, c0:c1], wv_t[:, c0:c1])
                    nc.sync.dma_start(wq_sb[0][:, c0:c1], wq_t[0][:, c0:c1])
                if d == 4:
                    nc.sync.dma_start(cc_sb[:], cc_d[:])
                    nc.sync.dma_start(ns_sb[:], ns_d[:])
                    nc.sync.dma_start(ident_sb[:], ident_d[:])
                    nc.sync.dma_start(emaskd_sb[:], emaskd_d[:])
                if 8 <= d < 20 and d % 2 == 0:
                    g, half = 1 + (d - 8) // 4, (d // 2) % 2
                    nc.sync.dma_start(wq_sb[g][:, 2048 * half:2048 * (half + 1)],
                                      wq_t[g][:, 2048 * half:2048 * (half + 1)])
                dl = 128 * d
                for sh in range(2):
                    nc.tensor.matmul(k_ps[sh][:], wk_sb[:, dl:dl + 128],
                                     xsl(d, sh), start=(d == 0), stop=(d == DT - 1))
                    nc.tensor.matmul(v_ps[sh][:], wv_sb[:, dl:dl + 128],
                                     xsl(d, sh), start=(d == 0), stop=(d == DT - 1))
                    nc.tensor.matmul(q0_ps[sh][:], wq_sb[0][:, dl:dl + 128],
                                     xsl(d, sh), start=(d == 0), stop=(d == DT - 1))

            nc.scalar.copy(v_et[:, 0:512], v_ps[0][:])
            nc.scalar.copy(v_et[:, 512:1024], v_ps[1][:])
            rope(k_ps[0], k_rot[:, 0:512], 0, fast=True)
            rope(k_ps[1], k_rot[:, 512:1024], 512, fast=True)
            rope(q0_ps[0], q_rot[0][:, 0:512], 0)
            rope(q0_ps[1], q_rot[0][:, 512:1024], 512)

            if phases < 2:
                for t in range(TT):
                    v_transpose(t)
                nc.sync.dma_start(outT[0:128, :], k_rot[:])
                return

            # ---- drip-fed Q1..Q3 projections ----
            qg_ps = {}
            jobs = deque()
            roped = {0}
            for g in range(1, NREP):
                for d in range(DT):
                    jobs.append(('q', g, d))
                jobs.append(('rope', g))

            def drip(n):
                while n > 0 and jobs:
                    job = jobs.popleft()
                    if job[0] == 'q':
                        _, g, d = job
                        if g not in qg_ps:
                            qg_ps[g] = [ps.tile([128, 512], f32, tag="ps",
                                                name=f"q{g}_ps{sh}")
                                        for sh in range(2)]
                        dl = 128 * d
                        for sh in range(2):
                            nc.tensor.matmul(qg_ps[g][sh][:],
                                             wq_sb[g][:, dl:dl + 128],
                                             xsl(d, sh),
                                             start=(d == 0), stop=(d == DT - 1))
                        n -= 2
                    else:
                        _, g = job
                        rope(qg_ps[g][0], q_rot[g][:, 0:512], 0, fast=True)
                        rope(qg_ps[g][1], q_rot[g][:, 512:1024], 512, fast=True)
                        roped.add(g)

            drip(8)
            for t in range(TT):
                v_transpose(t)

            # ---- attention per head, Q/wo work dripped between waves ----
            att_t = []
            inv_sqrt_hd = float(1.0 / np.sqrt(HD))
            wo_sb = []
            WAVES = [[0, 1], [2, 3], [4, 5], [6, 7]]
            pre_ops = []          # phase-C continuation of pre-accumulated wo

            for h in range(NREP if phases >= 3 else 0):
                # head h's q_rot must be fully emitted (incl. rope) before
                # its first scores land in the in-order PE queue
                while h not in roped:
                    drip(8)
                if h == 2 and phases >= 4:
                    for g in range(NREP):
                        w = wts.tile([128, D], bf16, name=f"wo_sb{g}", tag="w16")
                        nc.sync.dma_start(w[:], wo_t[128 * g:128 * (g + 1), :])
                        wo_sb.append(w)

                def emit_sc_exp(t):
                    dlo, dhi = 128 * t, 128 * (t + 1)
                    expm = epool.tile([128, S], bf16, name="expm")
                    for c in range(2):
                        sc = ps.tile([128, 512], f32, tag="ps", name="sc")
                        nc.tensor.matmul(sc[:], k_rot[:, dlo:dhi],
                                         q_rot[h][:, 512 * c:512 * (c + 1)],
                                         start=True, stop=True)
                        lo, hi = 512 * c, 512 * (c + 1)
                        if dlo >= hi:
                            nc.scalar.activation(expm[:, lo:hi], sc[:], Exp,
                                                 scale=inv_sqrt_hd)
                        elif dhi <= lo:
                            nc.scalar.activation(expm[:, lo:hi], sc[:], Exp,
                                                 scale=inv_sqrt_hd, bias=1.0)
                        else:
                            nc.scalar.activation(expm[:, lo:hi], sc[:], Exp,
                                                 scale=inv_sqrt_hd)
                            nc.gpsimd.tensor_tensor(
                                expm[:, dlo:dhi], expm[:, dlo:dhi],
                                emaskd_sb[:, 128 * t:128 * (t + 1)], op=mult)
                            if dhi < hi:
                                nc.gpsimd.tensor_scalar_mul(
                                    expm[:, dhi:hi], expm[:, dhi:hi],
                                    float(np.e))
                    return expm

                pend = [emit_sc_exp(0), emit_sc_exp(1)]
                expms = []
                oz_w = [ps.tile([128, 512], f32, tag="ps", name=f"oz{h}_{c}")
                        for c in WAVES[0]]
                for t in range(TT):
                    if t + 2 < TT:
                        pend.append(emit_sc_exp(t + 2))
                    expm_t = pend.pop(0)
                    expms.append(expm_t)
                    for gi, c in enumerate(WAVES[0]):
                        nc.tensor.matmul(oz_w[gi][:, 0:129],
                                         expm_t[:, 128 * c:128 * (c + 1)],
                                         v_te[t][:, 0:129],
                                         start=(t == 0), stop=(t == TT - 1))
                    drip(4)
                a = hs.tile([128, S], bf16, name=f"att_t{h}", tag="hs")

                def norm_chunk(c, oz):
                    z_sb = zpool.tile([128, 1], f32, name="z_sb")
                    nc.vector.tensor_scalar_add(z_sb[:], oz[:, 128:129],
                                                float(MAXSEQ - S))
                    rz = zpool.tile([128, 1], f32, name="rz")
                    nc.vector.reciprocal(rz[:], z_sb[:])
                    att_n = apool.tile([128, 128], bf16, name="att_n")
                    nc.vector.tensor_scalar_mul(att_n[:], oz[:, 0:128], rz[:])
                    return att_n

                def tr_chunk(c, att_n):
                    tr = ps.tile([128, 128], bf16, tag="ps", name="tr")
                    nc.tensor.transpose(tr[:], att_n[:], ident_sb[:])
                    if c % 2 == 0:
                        nc.scalar.copy(a[:, 128 * c:128 * (c + 1)], tr[:])
                    else:
                        nc.vector.tensor_copy(a[:, 128 * c:128 * (c + 1)], tr[:])

                prev = list(zip(WAVES[0], oz_w))
                for w in range(1, 5):
                    if w < 4:
                        oz_w = [ps.tile([128, 512], f32, tag="ps",
                                        name=f"oz{h}_{c}") for c in WAVES[w]]
                        for gi, c in enumerate(WAVES[w]):
                            for t in range(TT):
                                nc.tensor.matmul(oz_w[gi][:, 0:129],
                                                 expms[t][:, 128 * c:128 * (c + 1)],
                                                 v_te[t][:, 0:129],
                                                 start=(t == 0), stop=(t == TT - 1))
                    normed = [(c, norm_chunk(c, oz)) for c, oz in prev]
                    drip(6)
                    if h == NREP - 1 and phases >= 4 and 2 <= w < 2 + NPRE:
                        # pre-accumulate heads 0..2 of wo tile do=w-2
                        do = w - 2
                        op_ps = [ps.tile([128, 512], f32, tag="ps",
                                         name=f"opp{do}_{c}") for c in range(2)]
                        for g in range(NREP - 1):
                            for c in range(2):
                                nc.tensor.matmul(
                                    op_ps[c][:],
                                    wo_sb[g][:, 128 * do:128 * (do + 1)],
                                    att_t[g][:, 512 * c:512 * (c + 1)],
                                    start=(g == 0), stop=False)
                        pre_ops.append((do, op_ps))
                    for c, att_n in normed:
                        tr_chunk(c, att_n)
                    prev = list(zip(WAVES[w], oz_w)) if w < 4 else []
                att_t.append(a)

            if phases == 3:
                for h in range(NREP):
                    nc.sync.dma_start(outT[128 * h:128 * (h + 1), :], att_t[h][:])
                return

            # ---- phase C: output projection ----
            def emit_out(do, op_ps):
                out_sb = opool.tile([128, S], bf16, name="out_sb")
                nc.vector.tensor_copy(out_sb[:, 0:512], op_ps[0][:])
                nc.scalar.copy(out_sb[:, 512:1024], op_ps[1][:])
                nc.sync.dma_start(outT[128 * do:128 * (do + 1), :], out_sb[:])

            for do, op_ps in pre_ops:
                for c in range(2):
                    nc.tensor.matmul(op_ps[c][:],
                                     wo_sb[NREP - 1][:, 128 * do:128 * (do + 1)],
                                     att_t[NREP - 1][:, 512 * c:512 * (c + 1)],
                                     start=False, stop=True)
                emit_out(do, op_ps)

            for do in range(len(pre_ops), DT if phases >= 4 else 0):
                op_ps = [ps.tile([128, 512], f32, tag="ps", name=f"op{c}")
                         for c in range(2)]
                for g in range(NREP):
                    for c in range(2):
                        nc.tensor.matmul(op_ps[c][:],
                                         wo_sb[g][:, 128 * do:128 * (do + 1)],
                                         att_t[g][:, 512 * c:512 * (c + 1)],
                                         start=(g == 0), stop=(g == NREP - 1))
                emit_out(do, op_ps)

        for _rep in range(repeat):
            _body()

    nc.compile()
    return nc


def kernel(**inputs):
    import ml_dtypes
    from concourse.bass_utils import run_bass_kernel_spmd
    bf = ml_dtypes.bfloat16

    x = np.asarray(inputs["x"], np.float32)                 # [1, S, D]
    cos = np.asarray(inputs["freqs_cos"], np.float32)       # [S, 64]
    sin = np.asarray(inputs["freqs_sin"], np.float32)       # [S, 64]
    wq = np.asarray(inputs["wq"], np.float32)               # [NH, HD, D]
    wk = np.asarray(inputs["wk"], np.float32)               # [NKV, HD, D]
    wv = np.asarray(inputs["wv"], np.float32)               # [NKV, HD, D]
    wo = np.asarray(inputs["wo"], np.float32)               # [D, D]
    input_pos = np.asarray(inputs["input_pos"]).astype(np.int64)  # [S]

    if "nc" not in _CACHE:
        _CACHE["nc"] = _build_nc()
    nc = _CACHE["nc"]

    perm = np.concatenate([np.arange(0, HD, 2), np.arange(1, HD, 2)])
    # xp: [128, DT*S], col d*1024 + sh*512 + s  (partition = d_within_tile)
    xT = x[0].T                                             # [D, S]
    xp = np.ascontiguousarray(
        xT.reshape(DT, 128, S).transpose(1, 0, 2).reshape(128, DT * S)).astype(bf)
    cc = np.ascontiguousarray(np.concatenate([cos.T, cos.T], 0))   # [128, S]
    ns = np.ascontiguousarray(np.concatenate([-sin.T, sin.T], 0))  # [128, S]
    emaskd_t = np.empty((TT, 128, 128), np.float32)
    for t in range(TT):
        p = input_pos[128 * t:128 * (t + 1)]
        emaskd_t[t] = np.where(p[:, None] <= p[None, :], np.float32(np.e),
                               np.float32(1.0))
    emaskd = np.ascontiguousarray(
        emaskd_t.transpose(1, 0, 2).reshape(128, TT * 128)).astype(bf)
    ident = np.eye(128, dtype=np.float32).astype(bf)

    def pmajor(wT):
        return np.ascontiguousarray(
            wT.reshape(DT, 128, HD).transpose(1, 0, 2).reshape(128, DT * HD))

    in_maps = []
    for g in range(NCORES):
        wq_g = wq[NREP * g:NREP * (g + 1)][:, perm, :]       # [4, 128, D]
        in_maps.append({
            "xp": xp,
            "wq_t": np.stack([pmajor(wq_g[j].T) for j in range(NREP)]).astype(bf),
            "wk_t": pmajor(wk[g][perm].T).astype(bf),        # [128, DT*128]
            "wv_t": pmajor(wv[g].T).astype(bf),              # [128, DT*128]
            "wo_t": np.ascontiguousarray(
                wo[:, NREP * HD * g:NREP * HD * (g + 1)].T).astype(bf),  # [512, D]
            "cc": cc, "ns": ns, "emaskd": emaskd, "ident": ident,
        })

    res = run_bass_kernel_spmd(nc, in_maps, list(range(NCORES)))
    total = np.zeros((D, S), np.float64)
    for g in range(NCORES):
        total += res.results[g]["outT"].astype(np.float64)
    return np.ascontiguousarray(total.T.astype(np.float32)[None])   # [1, S, D]
